# revision 1
# baseline (speedup 1.0000x reference)
"""Trainium2 Bass kernel for causal self-attention with segment masking.

Sharding: 8 cores = 2 batches x 4 head-groups (4 heads each).
Per core: QKV projection (bf16), S^T-layout attention with data-dependent
tile skipping (causal + segment structure), output projection producing a
partial [T, D] sum; host adds the 4 partials per batch.

Layouts (per core):
  x_T   [D, T]      bf16  (host-transposed)
  q_T/k_T [128, T]  bf16  two tiles, one per head pair (2 heads x 64 dims)
  v_ext [128, 16kb, 4h, 65] bf16 (col 64 = ones -> softmax denominator)
  s_T   [128k, 512q] f32 PSUM  (scores transposed; QK^T row-packed 2 heads)
  p_T   [128, 512]  bf16 SBUF = exp(s/8) * mask01
  y_ps  [65, 512]   f32 PSUM = v_ext.T @ p_T (row 64 = sum of p = denom)
  y_T   [128, 2, T] bf16 (normalized, feeds proj as lhsT)
"""

import numpy as np
import ml_dtypes

import concourse.bass as bass
import concourse.mybir as mybir
import concourse.tile as tile
from concourse import bacc
from concourse import bass_utils

B, T, D = 2, 2048, 1024
H, HD = 16, 64
QC = 512            # q chunk (matmul free dim)
KB = 128            # k block (partition dim)
NQC = T // QC       # 4
NKB = T // KB       # 16
DK = D // 128       # 8 contraction chunks for projections
BF16 = mybir.dt.bfloat16
F32 = mybir.dt.float32
nbf = ml_dtypes.bfloat16
Exp = mybir.ActivationFunctionType.Exp


def _schedule(seg):
    """Data-dependent tile schedule, shared (union) across both batches.

    Returns (tiles, mask_arrs, n_masked):
      tiles: list of (qc, kb, mask_idx) with mask_idx == -1 when the
             [128k x 512q] tile is fully allowed in BOTH batches.
      mask_arrs: per-batch packed bf16 {0,1} mask tiles [n_masked, KB, QC]
                 in mask_idx order (transposed layout: [k, q]).
    """
    ar = np.arange(T)
    masks = [
        (seg[b][:, None] == seg[b][None, :]) & (ar[:, None] <= ar[None, :])
        for b in range(B)
    ]  # mask_T[k, q]
    tiles = []
    mask_data = [[] for _ in range(B)]
    n_masked = 0
    for qc in range(NQC):
        for kb in range(NKB):
            if kb * KB > qc * QC + QC - 1:
                continue  # fully above the diagonal
            subs = [
                masks[b][kb * KB:(kb + 1) * KB, qc * QC:(qc + 1) * QC]
                for b in range(B)
            ]
            if not any(s.any() for s in subs):
                continue  # dead tile in both batches: skip entirely
            if all(s.all() for s in subs):
                tiles.append((qc, kb, -1))
            else:
                tiles.append((qc, kb, n_masked))
                for b in range(B):
                    mask_data[b].append(subs[b].astype(nbf))
                n_masked += 1
    if n_masked == 0:
        mask_arrs = [np.zeros((1, KB, QC), nbf) for _ in range(B)]
        n_masked = 1
    else:
        mask_arrs = [np.stack(mask_data[b]) for b in range(B)]
    return tiles, mask_arrs, n_masked


def _build(tiles, n_masked):
    nc = bacc.Bacc("TRN2", target_bir_lowering=False, debug=False, num_devices=8)
    xT = nc.dram_tensor("xT", [D, T], BF16, kind="ExternalInput").ap()
    wqkv = nc.dram_tensor("wqkv", [D, 768], BF16, kind="ExternalInput").ap()
    wp = nc.dram_tensor("wp", [256, D], BF16, kind="ExternalInput").ap()
    mk = nc.dram_tensor("mask", [n_masked, KB, QC], BF16, kind="ExternalInput").ap()
    out = nc.dram_tensor("out", [T, D], BF16, kind="ExternalOutput").ap()

    act = {qc: [] for qc in range(NQC)}
    for (qc, kb, mi) in tiles:
        act[qc].append((kb, mi))

    with tile.TileContext(nc) as tc:
        with (
            tc.tile_pool(name="const", bufs=1) as cpool,
            tc.tile_pool(name="work", bufs=7) as wpool,
            tc.tile_pool(name="psq", bufs=1, space="PSUM") as psq,
            tc.tile_pool(name="pss", bufs=2, space="PSUM") as pss,
            tc.tile_pool(name="psy", bufs=3, space="PSUM") as psy,
        ):
            # ---- input DMAs, interleaved so qkv can start early ----
            wqkv_sb = cpool.tile([128, DK, 768], BF16, tag="wqkv")
            x_sb = cpool.tile([128, DK, T], BF16, tag="x")
            for i in range(DK):
                eng = nc.sync if i % 2 == 0 else nc.scalar
                eng.dma_start(
                    x_sb[:, i, :],
                    xT[i * 128:(i + 1) * 128, :].rearrange("(o p) t -> p (o t)", p=128),
                )
                eng2 = nc.scalar if i % 2 == 0 else nc.sync
                eng2.dma_start(
                    wqkv_sb[:, i, :],
                    wqkv[i * 128:(i + 1) * 128, :].rearrange("(o p) n -> p (o n)", p=128),
                )
            mask_sb = cpool.tile([128, n_masked, QC], BF16, tag="m")
            nc.sync.dma_start(mask_sb[:], mk.rearrange("n p q -> p n q"))
            wp_sb = cpool.tile([128, 2, D], BF16, tag="wp")
            nc.scalar.dma_start(wp_sb[:], wp.rearrange("(c p) n -> p c n", p=128))

            q_sb = [cpool.tile([128, T], BF16, tag=f"q{p}", name=f"q{p}") for p in range(2)]
            k_sb = [cpool.tile([128, T], BF16, tag=f"k{p}", name=f"k{p}") for p in range(2)]
            v_sb = cpool.tile([128, NKB, 4, 65], BF16, tag="v")
            y_qc = [cpool.tile([128, 2, QC], BF16, tag=f"y{qc}", name=f"y{qc}") for qc in range(NQC)]
            nc.vector.memset(v_sb[:, :, :, 64], 1.0)

            # PE warm-up burn: junk matmuls on the first weight chunk while
            # x DMAs land, so the HAM clock-gate opens before real work.
            warm = psy.tile([65, 512], F32, tag="psy", name="warm")
            for _ in range(24):
                nc.tensor.matmul(
                    warm[0:64, :], wqkv_sb[:, 0, 0:64], wqkv_sb[:, 0, 0:512],
                    start=True, stop=True,
                )

            _burn_n = [0]

            def emit_burn(n):
                _burn_n[0] += 1
                bt = psq.tile([128, 512], F32, tag="psq", name=f"burn{_burn_n[0]}")
                for _ in range(n):
                    nc.tensor.matmul(
                        bt[:], wqkv_sb[:, 0, 0:128], wqkv_sb[:, 0, 0:512],
                        start=True, stop=True,
                    )

            # ---- per-qc pipeline, qkv shifted one qc late as PE filler ----
            def emit_qkv_qk(qc, pairs=(0, 1)):
                for p in pairs:
                    ps = psq.tile([128, 512], F32, tag="psq", name=f"q_{qc}_{p}")
                    for i in range(DK):
                        nc.tensor.matmul(
                            ps[:], wqkv_sb[:, i, p * 128:(p + 1) * 128],
                            x_sb[:, i, qc * 512:(qc + 1) * 512],
                            start=(i == 0), stop=(i == DK - 1),
                        )
                    nc.vector.tensor_copy(out=q_sb[p][:, qc * 512:(qc + 1) * 512], in_=ps[:])
                for p in pairs:
                    ps = psq.tile([128, 512], F32, tag="psq", name=f"k_{qc}_{p}")
                    for i in range(DK):
                        nc.tensor.matmul(
                            ps[:], wqkv_sb[:, i, 256 + p * 128:256 + (p + 1) * 128],
                            x_sb[:, i, qc * 512:(qc + 1) * 512],
                            start=(i == 0), stop=(i == DK - 1),
                        )
                    nc.vector.tensor_copy(out=k_sb[p][:, qc * 512:(qc + 1) * 512], in_=ps[:])

            def emit_qkv_v(qc, pairs=(0, 1)):
                for kb in range(qc * 4, qc * 4 + 4):
                    for p in pairs:
                        ps = psq.tile([128, 512], F32, tag="psq", name=f"v_{kb}_{p}")
                        for i in range(DK):
                            nc.tensor.matmul(
                                ps[:, :128], x_sb[:, i, kb * 128:(kb + 1) * 128],
                                wqkv_sb[:, i, 512 + p * 128:512 + (p + 1) * 128],
                                start=(i == 0), stop=(i == DK - 1),
                            )
                        nc.vector.tensor_copy(
                            out=v_sb[:, kb, p * 2:p * 2 + 2, 0:64],
                            in_=ps[:, :128].rearrange("p (h d) -> p h d", h=2),
                        )

            def emit_attn(qc, p):
                kbs = act[qc]
                y_ps = [psy.tile([65, 512], F32, tag="psy", name=f"yps{p}_{qc}_{hh}") for hh in range(2)]
                for idx, (kb, mi) in enumerate(kbs):
                    first, last = idx == 0, idx == len(kbs) - 1
                    s_ps = pss.tile([128, 1024], F32, tag="pss", name=f"s_{p}_{qc}_{kb}")
                    for hh in range(2):
                        lo = hh * 64
                        nc.tensor.matmul(
                            s_ps[:, hh * 512:(hh + 1) * 512],
                            k_sb[p][lo:lo + 64, kb * 128:(kb + 1) * 128],
                            q_sb[p][lo:lo + 64, qc * 512:(qc + 1) * 512],
                            start=True, stop=True,
                        )
                    pt = wpool.tile([128, 1024], BF16, tag="pt", name=f"pt{p}_{qc}_{kb}")
                    nc.scalar.activation(pt[:], s_ps[:], Exp, scale=0.125)
                    if mi >= 0:
                        nc.vector.tensor_tensor(
                            out=pt.rearrange("p (c q) -> p c q", c=2),
                            in0=pt.rearrange("p (c q) -> p c q", c=2),
                            in1=mask_sb[:, mi, None, :].to_broadcast((128, 2, QC)),
                            op=mybir.AluOpType.mult,
                        )
                    for hh in range(2):
                        nc.tensor.matmul(
                            y_ps[hh][:], v_sb[:, kb, p * 2 + hh, :],
                            pt[:, hh * 512:(hh + 1) * 512],
                            start=first, stop=last,
                        )
                for hh in range(2):
                    lr = wpool.tile([65, 512], F32, tag="lr")
                    lp = wpool.tile([128, 4], F32, tag="lp")
                    l0 = wpool.tile([1, 512], F32, tag="l0")
                    lb = wpool.tile([64, 512], F32, tag="lb")
                    nc.vector.tensor_copy(out=lr[64:65, :], in_=y_ps[hh][64:65, :])
                    nc.sync.dma_start(lp[:], lr[64:65, :])
                    nc.vector.reciprocal(lp[:], lp[:])
                    nc.sync.dma_start(l0[:], lp[:])
                    nc.gpsimd.partition_broadcast(lb[:], l0[:])
                    if hh == 0:
                        nc.vector.tensor_mul(
                            out=y_qc[qc][0:64, p, :],
                            in0=y_ps[hh][0:64, :], in1=lb[:],
                        )
                    else:
                        yt = wpool.tile([64, 512], BF16, tag="yt")
                        nc.vector.tensor_mul(out=yt[:], in0=y_ps[hh][0:64, :], in1=lb[:])
                        nc.sync.dma_start(y_qc[qc][64:128, p, :], yt[:])

            def emit_proj(qc):
                for mt in range(qc * 4, qc * 4 + 4):
                    ot = wpool.tile([128, 1024], BF16, tag="ot", name=f"ot{mt}")
                    for n in range(2):
                        ps = pss.tile([128, 1024], F32, tag="pss", name=f"pso{mt}_{n}")[:, :512]
                        for c in range(2):
                            nc.tensor.matmul(
                                ps[:], y_qc[qc][:, c, (mt % 4) * 128:(mt % 4) * 128 + 128],
                                wp_sb[:, c, n * 512:(n + 1) * 512],
                                start=(c == 0), stop=(c == 1),
                            )
                        nc.vector.tensor_copy(out=ot[:, n * 512:(n + 1) * 512], in_=ps[:])
                    nc.gpsimd.dma_start(out[mt * 128:(mt + 1) * 128, :], ot[:])

            emit_qkv_qk(0)
            emit_qkv_v(0)
            for qc in range(3):
                emit_attn(qc, 0)
                if qc < 2:
                    emit_qkv_qk(qc + 1)
                else:
                    emit_qkv_qk(3, pairs=(0,))
                    emit_qkv_v(3, pairs=(0,))
                emit_attn(qc, 1)
                if qc < 2:
                    emit_qkv_v(qc + 1)
            emit_attn(3, 0)
            emit_qkv_qk(3, pairs=(1,))
            emit_qkv_v(3, pairs=(1,))
            emit_proj(0)
            emit_attn(3, 1)
            emit_proj(1)
            emit_proj(2)
            emit_burn(10)
            emit_proj(3)

    nc.compile()
    return nc


def _in_maps(x, seg, Wqkv, Wproj, mask_arrs):
    maps = []
    for c in range(8):
        b, g = divmod(c, 4)
        h0 = g * 4
        cs, ce = h0 * 64, h0 * 64 + 256
        maps.append({
            "xT": np.ascontiguousarray(x[b].T).astype(nbf),
            "wqkv": np.ascontiguousarray(np.concatenate(
                [Wqkv[:, cs:ce], Wqkv[:, D + cs:D + ce], Wqkv[:, 2 * D + cs:2 * D + ce]],
                axis=1)).astype(nbf),
            "wp": np.ascontiguousarray(Wproj[cs:ce, :]).astype(nbf),
            "mask": mask_arrs[b],
        })
    return maps


_CACHE = {}


def _prepare(x, segment_ids, W_qkv, W_proj):
    x = np.asarray(x, np.float32)
    seg = np.asarray(segment_ids)
    Wqkv = np.asarray(W_qkv, np.float32)
    Wproj = np.asarray(W_proj, np.float32)
    tiles, mask_arrs, n_masked = _schedule(seg)
    key = (tuple(tiles), n_masked)
    if key not in _CACHE:
        _CACHE[key] = _build(tiles, n_masked)
    nc = _CACHE[key]
    return nc, _in_maps(x, seg, Wqkv, Wproj, mask_arrs)


def kernel(x, segment_ids, W_qkv, W_proj):
    nc, in_maps = _prepare(x, segment_ids, W_qkv, W_proj)
    res = bass_utils.run_bass_kernel_spmd(nc, in_maps, core_ids=list(range(8)))
    out = np.zeros((B, T, D), np.float32)
    for c in range(8):
        out[c // 4] += res.results[c]["out"].astype(np.float32)
    return out

